# revision 1
# baseline (speedup 1.0000x reference)
"""BiLSTM-CRF Trainium2 kernel (8-core SPMD, batch-sharded).

Per core: 4 sequences, full pipeline on device:
  embedding gather (indirect DMA) -> PE transposes -> input-gate GEMMs ->
  512-step bidirectional LSTM recurrence -> emission GEMM ->
  512-step Viterbi max-plus scan -> batched pointer extraction ->
  512-step backtrace -> int32 tag path.

All layout preparation (transposes / scaling / gate reordering) happens on the
host in numpy; all FLOPs and the memory-bound gather happen on device.

Math notes:
  sigmoid(x) = 0.5*tanh(0.5x)+0.5 so every gate uses one Tanh activation; the
  0.5 factors are pre-folded into the weights. Cell/hidden state are carried
  doubled (C=2c, H=2h) so the whole cell update is 4 fused
  scalar_tensor_tensor ops; the 0.5 for H is folded into W_hh and W_out.
"""

import numpy as np

import concourse.bass as bass
import concourse.tile as tile
from concourse import bacc, mybir
from concourse.bass_utils import run_bass_kernel_spmd

FP = mybir.dt.float32
I32 = mybir.dt.int32
AX = mybir.AxisListType
OP = mybir.AluOpType
AF = mybir.ActivationFunctionType

VOCAB = 100000
E = 256
Hh = 128
K = 12
START = 9
STOP = 10
NEG = -10000.0
B = 32
NCORES = 8
BL = B // NCORES  # 4 sequences per core


def build_program(T=512):
    nc = bacc.Bacc("TRN2", target_bir_lowering=False, debug=False)
    NTOK = T * BL              # tokens per core
    NTILE = NTOK // 128        # gather tiles (16 at T=512)
    NCHUNK = NTOK // 512       # 512-col GEMM chunks (4)

    def din(name, shape, dtype=FP):
        return nc.dram_tensor(name, list(shape), dtype, kind="ExternalInput").ap()

    idx_in = din("idx_in", [128, NTILE], I32)          # [p,k] token ids, time-major
    embed = din("embed", [VOCAB, E])
    w_ihT = din("w_ihT", [2, E, 4 * Hh])               # pre-scaled, gate order i,f,o,g
    w_hhT = din("w_hhT", [2, Hh, 4 * Hh])
    b_in = din("b_in", [128, 8])                       # col d*4+g: per-partition bias
    h_init = din("h_init", [2, 128, BL])               # 2*h0, feature-major
    c_init = din("c_init", [2, 128, BL])               # 2*c0
    w_outT = din("w_outT", [2, Hh, K])                 # 0.5*W_out halves, transposed
    bout_rep = din("bout_rep", [128, K])
    ident = din("ident", [128, 128])
    trans128 = din("trans128", [128, K * K])           # trans[j,k] flat, replicated
    wvec128 = din("wvec128", [128, K])                 # 11-k, replicated
    tstop = din("tstop", [BL, K])                      # trans[STOP,:] replicated
    scores0 = din("scores0", [BL, K])

    path_out = nc.dram_tensor("path_out", [BL, T], I32, kind="ExternalOutput").ap()

    # DRAM scratch for partition-permute bounces
    f128_d = nc.dram_tensor("f128_d", [NTILE, 128, K], FP).ap()
    s4_d = nc.dram_tensor("s4_d", [T + 1, BL, K], FP).ap()
    w128_d = nc.dram_tensor("w128_d", [128, NTILE * K], FP).ap()

    with tile.TileContext(nc) as tc:
        with tc.tile_pool(name="const", bufs=1) as cpool, \
             tc.tile_pool(name="big", bufs=1) as bpool:

            # ---- load constants ----
            def cload(ap_in, shape, dtype=FP):
                t = cpool.tile(list(shape), dtype, name=f"c_{ap_in.tensor.name if hasattr(ap_in,'tensor') else id(ap_in)}_{np.random.randint(1<<30)}")
                nc.sync.dma_start(t[:], ap_in)
                return t

            idx_sb = cload(idx_in, [128, NTILE], I32)
            wih_sb = [[cload(w_ihT[d, e * 128:(e + 1) * 128, :], [128, 4 * Hh])
                       for e in range(2)] for d in range(2)]
            whh_sb = [cload(w_hhT[d], [Hh, 4 * Hh]) for d in range(2)]
            b_sb = cload(b_in, [128, 8])
            hi_sb = [cload(h_init[d], [128, BL]) for d in range(2)]
            ci_sb = [cload(c_init[d], [128, BL]) for d in range(2)]
            wout_sb = [cload(w_outT[d], [Hh, K]) for d in range(2)]
            bout_sb = cload(bout_rep, [128, K])
            id_sb = cload(ident, [128, 128])
            tr_sb = cload(trans128, [128, K * K])
            wv_sb = cload(wvec128, [128, K])
            ts_sb = cload(tstop, [BL, K])
            s0_sb = cload(scores0, [BL, K])

            # big persistent arrays
            xg_sb = [bpool.tile([128, T * 16], FP, tag=f"xg{d}", name=f"xg{d}") for d in range(2)]
            hs_sb = [bpool.tile([128, T * BL], FP, tag=f"hs{d}", name=f"hs{d}") for d in range(2)]
            S_sb = bpool.tile([BL, T * K], FP, tag="S", name="S")
            ft4 = bpool.tile([BL, T * K], FP, tag="ft4", name="ft4")
            wptr4 = bpool.tile([BL, T * K], FP, tag="wptr4", name="wptr4")
            wpath = bpool.tile([BL, T], FP, tag="wpath", name="wpath")

            # ---- phase 1: embedding gather + transpose to [E, tok] ----
            with tc.tile_pool(name="gat", bufs=3) as gpool, \
                 tc.tile_pool(name="ps1", bufs=4, space="PSUM") as ps1, \
                 tc.tile_pool(name="xe", bufs=1) as xepool:
                xe_sb = [xepool.tile([128, NTOK], FP, tag=f"xe{e}", name=f"xe{e}") for e in range(2)]
                for k in range(NTILE):
                    gt = gpool.tile([128, E], FP)
                    nc.gpsimd.indirect_dma_start(
                        out=gt[:],
                        out_offset=None,
                        in_=embed[:],
                        in_offset=bass.IndirectOffsetOnAxis(
                            ap=idx_sb[:, k:k + 1], axis=0),
                    )
                    for e in range(2):
                        pt = ps1.tile([128, 128], FP, space="PSUM")
                        nc.tensor.transpose(
                            out=pt[:], in_=gt[:, e * 128:(e + 1) * 128],
                            identity=id_sb[:])
                        nc.vector.tensor_copy(
                            xe_sb[e][:, k * 128:(k + 1) * 128], pt[:])

                # ---- phase 2: xg = W_ih_eff @ xe + b, interleaved [t,(g,b)] ----
                with tc.tile_pool(name="ps2", bufs=3, space="PSUM") as ps2:
                    for d in range(2):
                        xgv = xg_sb[d][:].rearrange("p (t x) -> p t x", x=16)
                        for g in range(4):
                            for c in range(NCHUNK):
                                pt = ps2.tile([128, 512], FP, space="PSUM")
                                for e in range(2):
                                    nc.tensor.matmul(
                                        pt[:],
                                        lhsT=wih_sb[d][e][:, g * 128:(g + 1) * 128],
                                        rhs=xe_sb[e][:, c * 512:(c + 1) * 512],
                                        start=(e == 0), stop=(e == 1),
                                    )
                                nc.vector.tensor_scalar(
                                    out=xgv[:, c * 128:(c + 1) * 128,
                                            g * 4:(g + 1) * 4],
                                    in0=pt[:].rearrange("p (t b) -> p t b", b=BL),
                                    scalar1=b_sb[:, d * 4 + g:d * 4 + g + 1],
                                    scalar2=None,
                                    op0=OP.add,
                                )

            # ---- phase 3: LSTM recurrence, both directions interleaved ----
            # gate cols per step: i=0:4, f=4:8, o=8:12, g=12:16
            with tc.tile_pool(name="ps3", bufs=4, space="PSUM") as ps3, \
                 tc.tile_pool(name="th", bufs=4) as thpool, \
                 tc.tile_pool(name="cell", bufs=4) as cellpool, \
                 tc.tile_pool(name="cst", bufs=2) as cstpool:
                c_cur = [ci_sb[0], ci_sb[1]]
                for step in range(T):
                    tt = [step, T - 1 - step]
                    prev = [hi_sb[d][:] if step == 0 else
                            hs_sb[d][:, (tt[d] - 1 + 2 * d) * BL:
                                      (tt[d] + 2 * d) * BL]
                            for d in range(2)]
                    # stage-major emission: engine queues alternate f/r so a
                    # stalled instruction never blocks the other chain.
                    pt = []
                    for d in range(2):
                        p = ps3.tile([128, 16], FP, space="PSUM",
                                     tag=f"g{d}", name=f"g{d}_{step}")
                        pt.append(p)
                        for q in range(4):
                            nc.tensor.matmul(
                                p[32 * q:32 * (q + 1), :],
                                lhsT=id_sb[:, 32 * q:32 * (q + 1)],
                                rhs=xg_sb[d][:, tt[d] * 16:(tt[d] + 1) * 16],
                                start=True, stop=False,
                                tile_position=(0, 32 * q),
                                skip_group_check=True)
                    for d in range(2):
                        for g in range(4):
                            for q in range(4):
                                nc.tensor.matmul(
                                    pt[d][32 * q:32 * (q + 1), g * 4:(g + 1) * 4],
                                    lhsT=whh_sb[d][:, g * 128 + 32 * q:
                                                   g * 128 + 32 * (q + 1)],
                                    rhs=prev[d],
                                    start=False, stop=(g == 3 and q == 3),
                                    tile_position=(0, 32 * q),
                                    skip_group_check=True)
                    th = []
                    for d in range(2):
                        t_ = thpool.tile([128, 16], FP, tag=f"th{d}",
                                         name=f"th{d}_{step}")
                        th.append(t_)
                        nc.scalar.activation(t_[:], pt[d][:], AF.Tanh)
                    ab = []
                    for d in range(2):
                        a_t = cellpool.tile([128, BL], FP, tag=f"a{d}",
                                            name=f"a{d}_{step}")
                        b_t = cellpool.tile([128, BL], FP, tag=f"b{d}",
                                            name=f"b{d}_{step}")
                        nc.vector.scalar_tensor_tensor(
                            out=a_t[:], in0=th[d][:, 4:8], scalar=1.0,
                            in1=c_cur[d][:], op0=OP.add, op1=OP.mult)
                        nc.vector.scalar_tensor_tensor(
                            out=b_t[:], in0=th[d][:, 0:4], scalar=1.0,
                            in1=th[d][:, 12:16], op0=OP.add, op1=OP.mult)
                        ab.append((a_t, b_t))
                    c_new = []
                    for d in range(2):
                        c_n = cstpool.tile([128, BL], FP, tag=f"c{d}",
                                           name=f"c{d}_{step}")
                        c_new.append(c_n)
                        nc.vector.scalar_tensor_tensor(
                            out=c_n[:], in0=ab[d][0][:], scalar=0.5,
                            in1=ab[d][1][:], op0=OP.mult, op1=OP.add)
                    tc_t = []
                    for d in range(2):
                        t_ = cellpool.tile([128, BL], FP, tag=f"tc{d}",
                                           name=f"tc{d}_{step}")
                        tc_t.append(t_)
                        nc.scalar.activation(t_[:], c_new[d][:], AF.Tanh,
                                             scale=0.5)
                    for d in range(2):
                        nc.vector.scalar_tensor_tensor(
                            out=hs_sb[d][:, tt[d] * BL:(tt[d] + 1) * BL],
                            in0=th[d][:, 8:12], scalar=1.0,
                            in1=tc_t[d][:], op0=OP.add, op1=OP.mult)
                        c_cur[d] = c_new[d]

            # ---- phase 4: emission scores feats -> ft4 [BL, T*K] ----
            with tc.tile_pool(name="ps4", bufs=3, space="PSUM") as ps4, \
                 tc.tile_pool(name="fsb", bufs=3) as fpool:
                for ch in range(NTILE):
                    pt = ps4.tile([128, K], FP, space="PSUM")
                    for d in range(2):
                        nc.tensor.matmul(
                            pt[:],
                            lhsT=hs_sb[d][:, ch * 128:(ch + 1) * 128],
                            rhs=wout_sb[d][:],
                            start=(d == 0), stop=(d == 1))
                    fsb = fpool.tile([128, K], FP)
                    nc.vector.tensor_add(fsb[:], pt[:], bout_sb[:])
                    nc.sync.dma_start(f128_d[ch], fsb[:])
                nc.sync.dma_start(
                    ft4[:].rearrange("b (c tr j) -> b c tr j", c=NTILE, tr=32),
                    f128_d.rearrange("c (tr b) j -> b c tr j", b=BL),
                )

            # ---- phase 5: Viterbi max-plus scan (scores only) ----
            tr4 = tr_sb[0:BL, :].rearrange("b (j k) -> b j k", k=K)
            with tc.tile_pool(name="vit", bufs=1) as vpool:
                NROT = 4
                m_rot = [vpool.tile([BL, K * K], FP, name=f"m_{r}")
                         for r in range(NROT)]
                mx_rot = [vpool.tile([BL, K], FP, name=f"mx_{r}")
                          for r in range(NROT)]
                for t in range(T):
                    sprev = s0_sb[:] if t == 0 else S_sb[:, (t - 1) * K:t * K]
                    m_t = m_rot[t % NROT]
                    mx = mx_rot[t % NROT]
                    m3 = m_t[:].rearrange("b (j k) -> b j k", k=K)
                    nc.vector.tensor_tensor(
                        out=m3,
                        in0=sprev.unsqueeze(1).broadcast_to([BL, K, K]),
                        in1=tr4, op=OP.add)
                    nc.vector.reduce_max(mx[:], m3, axis=AX.X)
                    nc.vector.tensor_add(
                        S_sb[:, t * K:(t + 1) * K], mx[:],
                        ft4[:, t * K:(t + 1) * K])

            # ---- phase 6: batched pointer extraction ----
            # s4_d slot t holds S_{t-1} (slot 0 = scores0).
            nc.sync.dma_start(s4_d[0], s0_sb[:])
            nc.sync.dma_start(
                s4_d[1:].rearrange("t b j -> b t j"),
                S_sb[:].rearrange("b (t j) -> b t j", j=K))
            with tc.tile_pool(name="ptr", bufs=2) as ppool, \
                 tc.tile_pool(name="ptrw", bufs=1) as pwpool:
                # s128[p=(tr,b), (c,k)] = S_{t-1}[b, k] for t = c*32+tr
                s128 = pwpool.tile([128, NTILE * K], FP, tag="s128", name="s128")
                nc.sync.dma_start(
                    s128[:].rearrange("p (c j) -> p c j", j=K),
                    s4_d[0:T].rearrange("(c tr) b j -> c tr b j", tr=32)
                        .transpose([1, 2, 0, 3])
                        .rearrange("tr b c j -> (tr b) c j"),
                )
                wptr128 = pwpool.tile([128, NTILE * K], FP, tag="w128", name="w128")
                w3 = wptr128[:].rearrange("p (c j) -> p c j", j=K)
                for j in range(K):
                    mj = ppool.tile([128, NTILE * K], FP, tag="mj")
                    m3 = mj[:].rearrange("p (c k) -> p c k", k=K)
                    nc.vector.tensor_tensor(
                        out=m3,
                        in0=s128[:].rearrange("p (c k) -> p c k", k=K),
                        in1=tr_sb[:, j * K:(j + 1) * K].unsqueeze(1)
                            .broadcast_to([128, NTILE, K]),
                        op=OP.add)
                    mxj = ppool.tile([128, NTILE], FP, tag="mxj")
                    nc.vector.reduce_max(mxj[:], m3, axis=AX.X)
                    msk = ppool.tile([128, NTILE * K], FP, tag="msk")
                    nc.vector.tensor_tensor(
                        out=msk[:].rearrange("p (c k) -> p c k", k=K),
                        in0=m3,
                        in1=mxj[:].unsqueeze(2).broadcast_to([128, NTILE, K]),
                        op=OP.is_equal)
                    nc.vector.tensor_tensor(
                        out=msk[:].rearrange("p (c k) -> p c k", k=K),
                        in0=msk[:].rearrange("p (c k) -> p c k", k=K),
                        in1=wv_sb[:].unsqueeze(1).broadcast_to([128, NTILE, K]),
                        op=OP.mult)
                    nc.vector.reduce_max(
                        w3[:, :, j], msk[:].rearrange("p (c k) -> p c k", k=K),
                        axis=AX.X)
                nc.sync.dma_start(w128_d, wptr128[:])
                nc.sync.dma_start(
                    wptr4[:].rearrange("b (c tr j) -> b c tr j", c=NTILE, tr=32),
                    w128_d.rearrange("(tr b) (c j) -> b c tr j", b=BL, j=K),
                )

            # ---- phase 7: init best tag + backtrace ----
            with tc.tile_pool(name="bt", bufs=1) as btpool:
                fs = btpool.tile([BL, K], FP, tag="fs")
                nc.vector.tensor_add(fs[:], S_sb[:, (T - 1) * K:], ts_sb[:])
                mx8 = btpool.tile([BL, 8], FP, tag="mx8")
                nc.vector.max(mx8[:], fs[:])
                msk = btpool.tile([BL, K], FP, tag="bmsk")
                nc.vector.tensor_scalar(
                    out=msk[:], in0=fs[:], scalar1=mx8[:, 0:1], scalar2=None,
                    op0=OP.is_equal)
                nc.vector.tensor_mul(msk[:], msk[:], wv_sb[0:BL, :])
                nc.vector.reduce_max(wpath[:, T - 1:T], msk[:], axis=AX.X)
                oh = btpool.tile([BL, K], FP, tag="oh")
                scr = btpool.tile([BL, K], FP, tag="scr")
                nc.vector.tensor_scalar(
                    out=oh[:], in0=wv_sb[0:BL, :],
                    scalar1=wpath[:, T - 1:T], scalar2=None, op0=OP.is_equal)
                for t in range(T - 1, 0, -1):
                    nc.vector.scalar_tensor_tensor(
                        out=scr[:], in0=oh[:], scalar=1.0,
                        in1=wptr4[:, t * K:(t + 1) * K],
                        op0=OP.mult, op1=OP.mult,
                        accum_out=wpath[:, t - 1:t])
                    if t > 1:
                        nc.vector.tensor_scalar(
                            out=oh[:], in0=wv_sb[0:BL, :],
                            scalar1=wpath[:, t - 1:t], scalar2=None,
                            op0=OP.is_equal)

                # ---- phase 8: path = 11 - wpath -> int32 -> out ----
                pi = btpool.tile([BL, T], I32, tag="pi")
                nc.vector.tensor_scalar(
                    out=pi[:], in0=wpath[:], scalar1=-1.0, scalar2=float(K - 1),
                    op0=OP.mult, op1=OP.add)
                nc.sync.dma_start(path_out, pi[:])

    nc.compile()
    return nc


def prep_inputs(sentence, h0, c0, embed, W_ih_f, W_hh_f, b_f, W_ih_r, W_hh_r,
                b_r, W_out, b_out, transitions, T=512):
    """Host-side layout prep. Returns per-core input maps."""
    f32 = np.float32
    perm = np.r_[0:128, 128:256, 384:512, 256:384]  # i,f,g,o -> i,f,o,g
    gs = np.concatenate([np.full(128, s, f32) for s in (0.5, 0.5, 0.5, 1.0)])

    def prep_dir(W_ih, W_hh, b):
        Wi = np.asarray(W_ih, f32)[perm] * gs[:, None]
        bb = np.asarray(b, f32)[perm] * gs
        Wh = np.asarray(W_hh, f32)[perm] * (0.5 * gs)[:, None]
        return Wi.T.copy(), Wh.T.copy(), bb

    wihT_f, whhT_f, be_f = prep_dir(W_ih_f, W_hh_f, b_f)
    wihT_r, whhT_r, be_r = prep_dir(W_ih_r, W_hh_r, b_r)
    w_ihT = np.stack([wihT_f, wihT_r])
    w_hhT = np.stack([whhT_f, whhT_r])
    b_in = np.stack([be_f.reshape(4, 128), be_r.reshape(4, 128)])  # [2,4,128]
    b_in = b_in.reshape(8, 128).T.copy()                           # [128,8]

    Wo = np.asarray(W_out, f32) * 0.5
    w_outT = np.stack([Wo[:, :128].T.copy(), Wo[:, 128:].T.copy()])
    bout_rep = np.tile(np.asarray(b_out, f32)[None, :], (128, 1))

    tr = np.asarray(transitions, f32)
    trans128 = np.tile(tr.reshape(1, K * K), (128, 1))
    wvec128 = np.tile((K - 1 - np.arange(K, dtype=f32))[None, :], (128, 1))
    tstop = np.tile(tr[STOP][None, :], (BL, 1))
    s0 = np.full((BL, K), NEG, f32)
    s0[:, START] = 0.0
    ident = np.eye(128, dtype=f32)
    embed = np.asarray(embed, f32)
    sentence = np.asarray(sentence)

    maps = []
    for core in range(NCORES):
        sl = sentence[core * BL:(core + 1) * BL, :T].astype(np.int32)
        idx_tm = sl.T.reshape(-1)                       # n = t*BL+b
        idx_in = idx_tm.reshape(-1, 128).T.copy()       # [128, NTILE]
        h_i = 2.0 * np.asarray(h0, f32)[:, core * BL:(core + 1) * BL, :]
        c_i = 2.0 * np.asarray(c0, f32)[:, core * BL:(core + 1) * BL, :]
        maps.append({
            "idx_in": idx_in,
            "embed": embed,
            "w_ihT": w_ihT,
            "w_hhT": w_hhT,
            "b_in": b_in,
            "h_init": np.ascontiguousarray(h_i.transpose(0, 2, 1)),
            "c_init": np.ascontiguousarray(c_i.transpose(0, 2, 1)),
            "w_outT": w_outT,
            "bout_rep": bout_rep,
            "ident": ident,
            "trans128": trans128,
            "wvec128": wvec128,
            "tstop": tstop,
            "scores0": s0,
        })
    return maps


_NC_CACHE = {}


def kernel(sentence, h0, c0, embed, W_ih_f, W_hh_f, b_f, W_ih_r, W_hh_r, b_r,
           W_out, b_out, transitions):
    T = np.asarray(sentence).shape[1]
    if T not in _NC_CACHE:
        _NC_CACHE[T] = build_program(T)
    nc = _NC_CACHE[T]
    maps = prep_inputs(sentence, h0, c0, embed, W_ih_f, W_hh_f, b_f,
                       W_ih_r, W_hh_r, b_r, W_out, b_out, transitions, T=T)
    res = run_bass_kernel_spmd(nc, maps, list(range(NCORES)))
    out = np.concatenate([res.results[i]["path_out"] for i in range(NCORES)], axis=0)
    return out.astype(np.int32)

